# revision 51
# baseline (speedup 1.0000x reference)
"""Multi-head attention (B=4, S=2048, D=1024, H=16, causal) on 8 trn2 NeuronCores.

Sharding: tensor-parallel over heads. Core c owns heads {2c, 2c+1} = model dims
[c*128, (c+1)*128).

Per-core pipeline (all matmul inputs bf16, fp32 PSUM accumulation):
  A) Q/K/V projections in transposed layout  X_c [128 dims, rows]
     (lhsT = W.T chunk stationary, rhs = host-pretransposed input stream).
     f-outer loop order: each weight chunk loaded once per 2 psum groups.
  B) V transposed back to row-major via grouped PE transposes (4 per PSUM
     tile) + one merged 3D-AP DVE copy per destination segment into
     v_aug [128 k-rows, kblk, 192] = [h0 dims | ones | h1 dims], so each
     head's PV lhsT [dims 64 | ones 64] makes PSUM rows carry context +
     the softmax denominator replicated 64x.
  C) Attention per (batch, 512-q window, 128-k block), causal blocks only,
     software-pipelined (PV for block k emitted after scores for block
     k+4 so the PE never waits on the ACT exp): scoresT [k,q] via 2-head
     row-packed matmuls (column-trimmed on the diagonal), exp (scale=1/8
     folded in, no max subtraction - scores are O(1) by construction),
     triangular mask on diagonal blocks, PV accumulation per head.
     Softmax division: gather [l0|l1] -> reciprocal_approx_fast (full-tile
     DVE custom op) -> tensor_mul -> ctxT [dims, q] bf16.
  D) Output projection partials -> outT [1024 o, q] bf16 per window.
     Out-proj matmuls + evacuations are spread as fine-grained background
     items between attention blocks (evacs alternate ACT/DVE) so they
     never delay the EXP stream; host sums the 8 cores' partials in fp32,
     transposes, adds bo.
"""

import os
import sys
from collections import deque
from contextlib import ExitStack

sys.path.insert(0, "/opt/trn_rl_repo")

import numpy as np
import ml_dtypes

import concourse.bass as bass
import concourse.bacc as bacc
import concourse.mybir as mybir
import concourse.tile as tile
from concourse.bass_utils import run_bass_kernel_spmd

BF = mybir.dt.bfloat16
F32 = mybir.dt.float32
bf16 = ml_dtypes.bfloat16

B, S, D = 4, 2048, 1024
R = B * S  # 8192
NCORES = 8
QW = 512  # q-window
NKB = S // 128  # 16 k-blocks per batch

_CACHE: dict = {}


def _build_program() -> bass.Bass:
    nc = bacc.Bacc(None, num_devices=NCORES)
    # host pre-shuffled to [B, half, p, f, c] so one projection-half stages
    # as 128 contiguous 16KB runs (8x fewer DMA descriptors than the
    # row-strided [D, R] layout).
    xq6 = nc.dram_tensor("xq6", [B, 2, 128, 8, 1024], BF, kind="ExternalInput")
    xk6 = nc.dram_tensor("xk6", [B, 2, 128, 8, 1024], BF, kind="ExternalInput")
    xv6 = nc.dram_tensor("xv6", [B, 2, 128, 8, 1024], BF, kind="ExternalInput")
    # batch-0 k/v quarters 0-1 in their own contiguous layout so window 0
    # can start after only ~4MB (q half + k/v quarter 0) has streamed
    xb0 = nc.dram_tensor("xb0", [2, 2, 128, 8, 512], BF, kind="ExternalInput")
    # host pre-rearranged to [128, 8, 128] so the load is one contiguous DMA
    wq = nc.dram_tensor("wq", [128, 8, 128], BF, kind="ExternalInput")
    wk = nc.dram_tensor("wk", [128, 8, 128], BF, kind="ExternalInput")
    wv = nc.dram_tensor("wv", [128, 8, 128], BF, kind="ExternalInput")
    wo = nc.dram_tensor("wo", [128, D], BF, kind="ExternalInput")
    bq = nc.dram_tensor("bq", [128, 1], F32, kind="ExternalInput")
    bk = nc.dram_tensor("bk", [128, 1], F32, kind="ExternalInput")
    bv = nc.dram_tensor("bv", [128, 1], F32, kind="ExternalInput")
    tri = nc.dram_tensor("tri", [128, 2, 128], BF, kind="ExternalInput")
    ident = nc.dram_tensor("ident", [128, 128], BF, kind="ExternalInput")
    out_ext = nc.dram_tensor("out", [D, R], BF, kind="ExternalOutput")

    EXP = mybir.ActivationFunctionType.Exp

    with ExitStack() as ctx:
        tc = ctx.enter_context(tile.TileContext(nc))
        singles = ctx.enter_context(tc.tile_pool(name="singles", bufs=1))
        stage = ctx.enter_context(tc.tile_pool(name="stage", bufs=5))
        qkv = ctx.enter_context(tc.tile_pool(name="qkv", bufs=2))
        vst = ctx.enter_context(tc.tile_pool(name="vst", bufs=3))
        exps = ctx.enter_context(tc.tile_pool(name="exps", bufs=8))
        divp = ctx.enter_context(tc.tile_pool(name="divp", bufs=4))
        outp = ctx.enter_context(tc.tile_pool(name="outp", bufs=9))
        # PSUM budget (8 banks): io 2 (proj chains / out-proj / grouped V
        # transposes, all 2KB slots) + sc 4 (two [128,2,512] tiles) + pv 2.
        ps_io = ctx.enter_context(tc.tile_pool(name="ps_io", bufs=2, space="PSUM"))
        ps_sc = ctx.enter_context(tc.tile_pool(name="ps_sc", bufs=2, space="PSUM"))
        ps_pv = ctx.enter_context(tc.tile_pool(name="ps_pv", bufs=2, space="PSUM"))

        # resident constants.  Only wq/bq are needed by the first matmuls;
        # issue them first so the ring backlog ahead of the q data is tiny,
        # and defer the rest behind the first input half.
        wq_sb = singles.tile([128, 8, 128], BF, name="wq_sb")
        wk_sb = singles.tile([128, 8, 128], BF, name="wk_sb")
        wv_sb = singles.tile([128, 8, 128], BF, name="wv_sb")
        wo_sb = singles.tile([128, D], BF, name="wo_sb")
        bq_sb = singles.tile([128, 1], F32, name="bq_sb")
        bk_sb = singles.tile([128, 1], F32, name="bk_sb")
        bv_sb = singles.tile([128, 1], F32, name="bv_sb")
        tri_sb = singles.tile([128, 2, 128], BF, name="tri_sb")
        id_sb = singles.tile([128, 128], BF, name="id_sb")
        nc.sync.dma_start(wq_sb[:], wq[:, :, :])
        nc.sync.dma_start(bq_sb[:], bq[:, :])

        def emit_late_consts():
            nc.sync.dma_start(wk_sb[:], wk[:, :, :])
            nc.gpsimd.dma_start(wv_sb[:], wv[:, :, :])
            nc.sync.dma_start(wo_sb[:], wo[:, :])
            nc.gpsimd.dma_start(bk_sb[:], bk[:, :])
            nc.sync.dma_start(bv_sb[:], bv[:, :])
            nc.gpsimd.dma_start(tri_sb[:], tri[:, :, :])
            nc.sync.dma_start(id_sb[:], ident[:, :])

        warm_sb = singles.tile([128, 512], BF, name="warm_sb")
        nc.vector.memset(warm_sb[:], 0.0)
        # 14 warm matmuls (~6us cold) bridge the PE across the initial input
        # stream so HAM is at full clock when the first projections land
        warm_ps = ps_io.tile([128, 512], F32, tag="proj", name="warm_ps")
        for wi in range(14):
            nc.tensor.matmul(
                warm_ps[:],
                warm_sb[:, 0:128],
                warm_sb[:],
                start=(wi == 0),
                stop=(wi == 13),
            )

        tiles = {}
        bg = deque()  # background PE/evac work items (closures)

        def bg_tick(n=1):
            for _ in range(n):
                if not bg:
                    return
                bg.popleft()()

        def alloc_batch(b):
            q_sb = qkv.tile([128, S], BF, tag="q_sb", name=f"q_sb{b}")
            k_sb = qkv.tile([128, S], BF, tag="k_sb", name=f"k_sb{b}")
            # [h0 dims (0:64) | ones (64:128) | h1 dims (128:192)]
            v_aug = qkv.tile([128, NKB, 192], BF, tag="v_aug", name=f"v_aug{b}")
            nc.vector.memset(v_aug[:, :, 64:128], 1.0)
            tiles[b] = (q_sb, k_sb, v_aug)

        def emit_projection_dmas(b, which, halves=(0, 1), st=None, nchunks=2):
            # which: 0=q, 1=k, 2=v.  One projection-half stages as a single
            # [128, 8, 1024] super-tile, loaded by two dma_starts (f 0:4 and
            # 4:8, on different queue engines) whose source runs are 8KB
            # contiguous per partition -- the matmuls for f<4 can start as
            # soon as the first dma lands (subtile deps).
            if b not in tiles:
                alloc_batch(b)
            x6 = (xq6, xk6, xv6)[which]
            if st is None:
                st = {}
            for half in halves:
                s_t = stage.tile([128, 8, 1024], BF, tag="stage")
                for c in range(nchunks):
                    fw = 8 // nchunks
                    eng = nc.sync if c % 2 == 0 else nc.gpsimd
                    eng.dma_start(
                        s_t[:, c * fw : (c + 1) * fw, :],
                        x6[b, half, :, c * fw : (c + 1) * fw, :],
                    )
                st[half] = s_t
            return st

        def emit_pst_group(b, t, v_st):
            # Grouped V transpose: 4 PE transposes into one half-bank PSUM
            # tile, then 2 merged 3D-AP copies into v_aug. Runs as one
            # background item so it occupies an io slot briefly and once.
            _, _, v_aug = tiles[b]
            pst = ps_io.tile([128, 4, 128], BF, tag="proj", name=f"pst{b}_{t}")
            for s4 in range(4):
                nc.tensor.transpose(
                    pst[:, s4, :], v_st[:, s4 * 128 : (s4 + 1) * 128], id_sb[:]
                )
            nc.vector.tensor_copy(v_aug[:, 4 * t : 4 * t + 4, 0:64], pst[:, :, 0:64])
            nc.vector.tensor_copy(
                v_aug[:, 4 * t : 4 * t + 4, 128:192], pst[:, :, 64:128]
            )

        def emit_proj_evac(b, which, t, ps, inline_pst):
            q_sb, k_sb, v_aug = tiles[b]
            if which == 0:
                nc.vector.tensor_scalar_add(
                    q_sb[:, t * 512 : (t + 1) * 512], ps[:], bq_sb[:]
                )
            elif which == 1:
                nc.vector.tensor_scalar_add(
                    k_sb[:, t * 512 : (t + 1) * 512], ps[:], bk_sb[:]
                )
            else:
                v_st = vst.tile([128, 512], BF, tag="v_st")
                nc.vector.tensor_scalar_add(v_st[:], ps[:], bv_sb[:])
                if inline_pst:
                    emit_pst_group(b, t, v_st)
                else:
                    bg.append(lambda b=b, t=t, v_st=v_st: emit_pst_group(b, t, v_st))

        def emit_projection_quarter(b, which, t, s_t, inline_pst=False):
            # one [128, 8, 512] staged quarter -> one 8-matmul chain + evac
            w_sb = (wq_sb, wk_sb, wv_sb)[which]
            ps = ps_io.tile([128, 512], F32, tag="proj", name=f"psq{b}_{t}_{which}")
            for f in range(8):
                nc.tensor.matmul(
                    ps[:],
                    w_sb[:, f, :],
                    s_t[:, f, :],
                    start=(f == 0),
                    stop=(f == 7),
                )
            emit_proj_evac(b, which, t, ps, inline_pst)

        def emit_projection_mms(b, which, st, halves=(0, 1), inline_pst=False):
            # Emits the 2x2 psum groups + evac, consuming staged tiles.
            w_sb = (wq_sb, wk_sb, wv_sb)[which]

            def evac(t, ps):
                emit_proj_evac(b, which, t, ps, inline_pst)

            for half in halves:
                ps0 = ps_io.tile(
                    [128, 512], F32, tag="proj", name=f"ps{b}_{half}a_{which}"
                )
                ps1 = ps_io.tile(
                    [128, 512], F32, tag="proj", name=f"ps{b}_{half}b_{which}"
                )
                s_t = st[half]
                for f in range(8):
                    nc.tensor.matmul(
                        ps0[:],
                        w_sb[:, f, :],
                        s_t[:, f, 0:512],
                        start=(f == 0),
                        stop=(f == 7),
                    )
                    nc.tensor.matmul(
                        ps1[:],
                        w_sb[:, f, :],
                        s_t[:, f, 512:1024],
                        start=(f == 0),
                        stop=(f == 7),
                    )
                evac(half * 2, ps0)
                evac(half * 2 + 1, ps1)

        ot_pend = {}  # ob -> ot super-tile holding the even window's chunk

        def emit_po_item(ctx_t, win, ob):
            # one out-projection chunk: matmul + evac; the store DMA fires
            # once per window PAIR ([128, 1024] contiguous columns -> 2KB
            # descriptor runs, half the descriptor load).  Evacs split
            # ACT/DVE; late batches are exp-saturated on ACT, so bias their
            # evacs toward the DVE.
            po = ps_io.tile([128, 512], F32, tag="proj", name=f"po_{win}_{ob}")
            nc.tensor.matmul(
                po[:],
                wo_sb[:, ob * 128 : (ob + 1) * 128],
                ctx_t[:],
                start=True,
                stop=True,
            )
            slot = (win // 512) % 2
            if slot == 0:
                ot = outp.tile([128, 2, 512], BF, tag="ot")
                ot_pend[ob] = ot
            else:
                ot = ot_pend.pop(ob)
            use_dve = (ob % 2 == 0) if win < 2 * S else (ob % 4 != 3)
            if use_dve:
                nc.vector.tensor_copy(ot[:, slot, :], po[:])
            else:
                nc.scalar.copy(ot[:, slot, :], po[:])
            if slot == 1:
                nc.sync.dma_start(
                    out_ext[ob * 128 : (ob + 1) * 128, win - 512 : win + 512],
                    ot[:, :, :],
                )

        def emit_attention_qb(b, qb):
            # Software-pipelined: PV for block k is emitted after scores for
            # block k+3, so the PE never waits on the ACT exp of block k and
            # the PV LDWEIGHTS (which inherits the exp-done semaphore wait)
            # can prefetch into the background weight buffer.
            # Background items (out-proj chunks of the previous window,
            # grouped V transposes of the next batch) are drained one per
            # block so they fill the PE's exp-paced slack without ever
            # bunching up in the ACT queue.
            q_sb, k_sb, v_aug = tiles[b]
            nk = 4 * qb + 4  # causal: k-blocks 0 .. 4qb+3
            pv0 = ps_pv.tile([128, 512], F32, tag="pv", name=f"pv0_{b}_{qb}")
            pv1 = ps_pv.tile([128, 512], F32, tag="pv", name=f"pv1_{b}_{qb}")
            ets = {}

            def emit_scores(kblk):
                r = kblk - 4 * qb
                q_lo = max(0, r * 128)
                sc = ps_sc.tile([128, 2, 512], F32, tag="sc")
                for h in range(2):
                    nc.tensor.matmul(
                        sc[:, h, q_lo:512],
                        k_sb[h * 64 : (h + 1) * 64, kblk * 128 : (kblk + 1) * 128],
                        q_sb[h * 64 : (h + 1) * 64, qb * 512 + q_lo : (qb + 1) * 512],
                        start=True,
                        stop=True,
                        tile_position=(h * 64, 0),
                    )
                et = exps.tile([128, 2, 512], BF, tag="et")
                nc.scalar.activation(
                    et[:, :, q_lo:512], sc[:, :, q_lo:512], EXP, scale=0.125
                )
                if r >= 0:
                    nc.gpsimd.tensor_mul(
                        et[:, :, q_lo : q_lo + 128],
                        et[:, :, q_lo : q_lo + 128],
                        tri_sb[:],
                    )
                ets[kblk] = (et, q_lo)

            def emit_pv(kblk):
                et, q_lo = ets.pop(kblk)
                for h, pv in ((0, pv0), (1, pv1)):
                    nc.tensor.matmul(
                        pv[:, q_lo:512],
                        v_aug[:, kblk, h * 64 : h * 64 + 128],
                        et[:, h, q_lo:512],
                        start=(kblk == 0),
                        stop=(kblk == nk - 1),
                    )

            for kblk in range(nk):
                emit_scores(kblk)
                if kblk >= 4:
                    emit_pv(kblk - 4)
                bg_tick(1)
            emit_pv(nk - 4)
            emit_pv(nk - 3)
            emit_pv(nk - 2)
            emit_pv(nk - 1)

            # normalize: pv0 rows[0:64]=ctx~ h0, rows[64:128]=l h0 (replicated)
            #            pv1 rows[0:64]=l h1,   rows[64:128]=ctx~ h1
            # reciprocal_approx_fast only works on full-128-partition SBUF
            # tiles, so gather [l0 | l1] into lt first.
            lt = divp.tile([128, 512], F32, tag="lt")
            rec = divp.tile([128, 512], F32, tag="rec")
            ctx_t = divp.tile([128, 512], BF, tag="ctx_t")
            nc.vector.tensor_copy(lt[0:64, :], pv0[64:128, :])
            nc.vector.tensor_copy(lt[64:128, :], pv1[0:64, :])
            nc.vector.reciprocal_approx_fast(rec[:, :], lt[:, :])
            nc.vector.tensor_mul(ctx_t[0:64, :], pv0[0:64, :], rec[0:64, :])
            nc.vector.tensor_mul(ctx_t[64:128, :], pv1[64:128, :], rec[64:128, :])
            win = b * S + qb * 512
            for ob in range(8):
                bg.append(
                    lambda ctx_t=ctx_t, win=win, ob=ob: emit_po_item(ctx_t, win, ob)
                )

        # Bootstrap batch 0 by halves so attention starts as early as
        # possible: load+project half 0 of q/k/v (rows 0:1024 -> enough for
        # windows 0-1), then run windows 0-1 while half 1 loads and projects
        # as background items.  QKV(b+1) then interleaves with attention(b)
        # at qb granularity as usual.
        # round-robin the q/k/v half-0 chunk loads so each lands roughly in
        # consumption order (q chains first, then k, then v) instead of k/v
        # trailing the entire q stream in the rings
        alloc_batch(0)
        # q half-0 (2MB), then k/v quarters 0-1 from the contiguous xb0
        # layout (1MB each): window 0 starts after ~4MB has streamed,
        # window 1 after ~6MB, instead of waiting for all three halves.
        boot_q = stage.tile([128, 8, 1024], BF, tag="stage", name="boot_q")
        nc.sync.dma_start(boot_q[:, 0:4, :], xq6[0, 0, :, 0:4, :])
        nc.gpsimd.dma_start(boot_q[:, 4:8, :], xq6[0, 0, :, 4:8, :])
        bkv = {}
        qi = 0
        for t in range(2):
            for kv in range(2):
                s_t = stage.tile([128, 8, 512], BF, tag="bstage", bufs=4)
                eng = nc.sync if qi % 2 == 0 else nc.gpsimd
                qi += 1
                eng.dma_start(s_t[:], xb0[kv, t, :, :, :])
                bkv[(kv, t)] = s_t
        emit_late_consts()
        st1 = {}
        for which in range(3):
            st1[which] = emit_projection_dmas(0, which, halves=(1,))
        emit_projection_mms(0, 0, {0: boot_q}, halves=(0,), inline_pst=True)
        emit_projection_quarter(0, 1, 0, bkv[(0, 0)], inline_pst=True)
        emit_projection_quarter(0, 2, 0, bkv[(1, 0)], inline_pst=True)
        for which in range(3):
            bg.append(
                lambda which=which: emit_projection_mms(
                    0, which, st1[which], halves=(1,)
                )
            )
        for b in range(B):
            for qb in range(S // QW):
                if b == 0 and qb == 1:
                    # k/v quarter 1 (rows 512:1024) right before window 1
                    emit_projection_quarter(0, 1, 1, bkv[(0, 1)], inline_pst=True)
                    emit_projection_quarter(0, 2, 1, bkv[(1, 1)], inline_pst=True)
                if b + 1 < B and qb < 3:
                    # issue next batch's stage DMAs a full window before
                    # their matmuls so mid-run chains never wait on the rings
                    st_next = emit_projection_dmas(b + 1, qb)
                emit_attention_qb(b, qb)
                if b == 0 and qb == 1:
                    # half-1 projections must be done before window 2 reads
                    # rows 1024:2048
                    while bg:
                        bg.popleft()()
                if b + 1 < B and qb < 3:
                    emit_projection_mms(b + 1, qb, st_next)
        while bg:
            bg.popleft()()

    return nc


def _prep_in_maps(inputs):
    q = np.ascontiguousarray(inputs["query"], dtype=np.float32).reshape(R, D)
    k = np.ascontiguousarray(inputs["key"], dtype=np.float32).reshape(R, D)
    v = np.ascontiguousarray(inputs["value"], dtype=np.float32).reshape(R, D)
    Wq = np.asarray(inputs["Wq"], np.float32)
    Wk = np.asarray(inputs["Wk"], np.float32)
    Wv = np.asarray(inputs["Wv"], np.float32)
    Wo = np.asarray(inputs["Wo"], np.float32)
    bq = np.asarray(inputs["bq"], np.float32)
    bk = np.asarray(inputs["bk"], np.float32)
    bv = np.asarray(inputs["bv"], np.float32)

    def to6(x):
        # [R, D] -> [B, half, p, f, c] with x6[b,h,p,f,c] = x.T[f*128+p,
        # b*2048+h*1024+c]; per-(b,h,p) runs are 16KB contiguous.
        xT = np.ascontiguousarray(x.T).astype(bf16)  # [D, R]
        return np.ascontiguousarray(
            xT.reshape(8, 128, 4, 2, 1024).transpose(2, 3, 1, 0, 4)
        )

    xq6 = to6(q)
    xk6 = to6(k)
    xv6 = to6(v)
    # batch-0 k/v quarters 0-1 (rows 0:1024) in their own contiguous layout
    xb0 = np.ascontiguousarray(
        np.stack(
            [
                np.stack([x6[0, 0, :, :, 0:512], x6[0, 0, :, :, 512:1024]])
                for x6 in (xk6, xv6)
            ]
        )
    )
    WqT = np.ascontiguousarray(Wq.T).astype(bf16)
    WkT = np.ascontiguousarray(Wk.T).astype(bf16)
    WvT = np.ascontiguousarray(Wv.T).astype(bf16)
    WoT = np.ascontiguousarray(Wo.T).astype(bf16)
    tri_m = np.arange(128)[:, None] <= np.arange(128)[None, :]
    tri_h = np.ascontiguousarray(
        np.broadcast_to(tri_m[:, None, :], (128, 2, 128))
    ).astype(bf16)
    id_h = np.eye(128, dtype=np.float32).astype(bf16)

    in_maps = []
    for c in range(NCORES):
        sl = slice(c * 128, (c + 1) * 128)
        in_maps.append(
            {
                "xq6": xq6,
                "xk6": xk6,
                "xv6": xv6,
                "xb0": xb0,
                "wq": np.ascontiguousarray(
                    WqT[:, sl].reshape(8, 128, 128).transpose(1, 0, 2)
                ),
                "wk": np.ascontiguousarray(
                    WkT[:, sl].reshape(8, 128, 128).transpose(1, 0, 2)
                ),
                "wv": np.ascontiguousarray(
                    WvT[:, sl].reshape(8, 128, 128).transpose(1, 0, 2)
                ),
                "wo": np.ascontiguousarray(WoT[sl, :]),
                "bq": np.ascontiguousarray(bq[sl].reshape(128, 1)),
                "bk": np.ascontiguousarray(bk[sl].reshape(128, 1)),
                "bv": np.ascontiguousarray(bv[sl].reshape(128, 1)),
                "tri": tri_h,
                "ident": id_h,
            }
        )
    return in_maps


def kernel(**inputs) -> np.ndarray:
    nc = _CACHE.get("nc")
    if nc is None:
        nc = _build_program()
        nc.finalize()  # Bacc legalization (register alloc, event-sem splitting)
        _CACHE["nc"] = nc
    in_maps = _prep_in_maps(inputs)
    trace = bool(int(os.environ.get("KERNEL_TRACE", "0")))
    res = run_bass_kernel_spmd(nc, in_maps, list(range(NCORES)), trace=trace)
    _CACHE["last"] = res
    acc = res.results[0]["out"].astype(np.float32)
    for c in range(1, NCORES):
        acc += res.results[c]["out"].astype(np.float32)
    full = acc.T + np.asarray(inputs["bo"], np.float32)[None, :]
    return np.ascontiguousarray(full).reshape(B, S, D)


# revision 52
# speedup vs baseline: 1.0147x; 1.0147x over previous
"""Multi-head attention (B=4, S=2048, D=1024, H=16, causal) on 8 trn2 NeuronCores.

Sharding: tensor-parallel over heads. Core c owns heads {2c, 2c+1} = model dims
[c*128, (c+1)*128).

Per-core pipeline (all matmul inputs bf16, fp32 PSUM accumulation):
  A) Q/K/V projections in transposed layout  X_c [128 dims, rows]
     (lhsT = W.T chunk stationary, rhs = host-pretransposed input stream).
     f-outer loop order: each weight chunk loaded once per 2 psum groups.
  B) V transposed back to row-major via grouped PE transposes (4 per PSUM
     tile) + one merged 3D-AP DVE copy per destination segment into
     v_aug [128 k-rows, kblk, 192] = [h0 dims | ones | h1 dims], so each
     head's PV lhsT [dims 64 | ones 64] makes PSUM rows carry context +
     the softmax denominator replicated 64x.
  C) Attention per (batch, 512-q window, 128-k block), causal blocks only,
     software-pipelined (PV for block k emitted after scores for block
     k+4 so the PE never waits on the ACT exp): scoresT [k,q] via 2-head
     row-packed matmuls (column-trimmed on the diagonal), exp (scale=1/8
     folded in, no max subtraction - scores are O(1) by construction),
     triangular mask on diagonal blocks, PV accumulation per head.
     Softmax division: gather [l0|l1] -> reciprocal_approx_fast (full-tile
     DVE custom op) -> tensor_mul -> ctxT [dims, q] bf16.
  D) Output projection partials -> outT [1024 o, q] bf16 per window.
     Out-proj matmuls + evacuations are spread as fine-grained background
     items between attention blocks (evacs alternate ACT/DVE) so they
     never delay the EXP stream; host sums the 8 cores' partials in fp32,
     transposes, adds bo.
"""

import os
import sys
from collections import deque
from contextlib import ExitStack

sys.path.insert(0, "/opt/trn_rl_repo")

import numpy as np
import ml_dtypes

import concourse.bass as bass
import concourse.bacc as bacc
import concourse.mybir as mybir
import concourse.tile as tile
from concourse.bass_utils import run_bass_kernel_spmd

BF = mybir.dt.bfloat16
F32 = mybir.dt.float32
bf16 = ml_dtypes.bfloat16

B, S, D = 4, 2048, 1024
R = B * S  # 8192
NCORES = 8
QW = 512  # q-window
NKB = S // 128  # 16 k-blocks per batch

_CACHE: dict = {}


def _build_program() -> bass.Bass:
    nc = bacc.Bacc(None, num_devices=NCORES)
    # host pre-shuffled to [B, half, p, f, c] so one projection-half stages
    # as 128 contiguous 16KB runs (8x fewer DMA descriptors than the
    # row-strided [D, R] layout).
    xq6 = nc.dram_tensor("xq6", [B, 2, 128, 8, 1024], BF, kind="ExternalInput")
    xk6 = nc.dram_tensor("xk6", [B, 2, 128, 8, 1024], BF, kind="ExternalInput")
    xv6 = nc.dram_tensor("xv6", [B, 2, 128, 8, 1024], BF, kind="ExternalInput")
    # batch-0 k/v quarters 0-1 in their own contiguous layout so window 0
    # can start after only ~4MB (q half + k/v quarter 0) has streamed
    xb0 = nc.dram_tensor("xb0", [2, 2, 128, 8, 512], BF, kind="ExternalInput")
    # host pre-rearranged to [128, 8, 128] so the load is one contiguous DMA
    wq = nc.dram_tensor("wq", [128, 8, 128], BF, kind="ExternalInput")
    wk = nc.dram_tensor("wk", [128, 8, 128], BF, kind="ExternalInput")
    wv = nc.dram_tensor("wv", [128, 8, 128], BF, kind="ExternalInput")
    wo = nc.dram_tensor("wo", [128, D], BF, kind="ExternalInput")
    bq = nc.dram_tensor("bq", [128, 1], F32, kind="ExternalInput")
    bk = nc.dram_tensor("bk", [128, 1], F32, kind="ExternalInput")
    bv = nc.dram_tensor("bv", [128, 1], F32, kind="ExternalInput")
    tri = nc.dram_tensor("tri", [128, 2, 128], BF, kind="ExternalInput")
    ident = nc.dram_tensor("ident", [128, 128], BF, kind="ExternalInput")
    out_ext = nc.dram_tensor("out", [D, R], BF, kind="ExternalOutput")

    EXP = mybir.ActivationFunctionType.Exp

    with ExitStack() as ctx:
        tc = ctx.enter_context(tile.TileContext(nc))
        singles = ctx.enter_context(tc.tile_pool(name="singles", bufs=1))
        stage = ctx.enter_context(tc.tile_pool(name="stage", bufs=5))
        qkv = ctx.enter_context(tc.tile_pool(name="qkv", bufs=2))
        vst = ctx.enter_context(tc.tile_pool(name="vst", bufs=3))
        exps = ctx.enter_context(tc.tile_pool(name="exps", bufs=8))
        divp = ctx.enter_context(tc.tile_pool(name="divp", bufs=4))
        outp = ctx.enter_context(tc.tile_pool(name="outp", bufs=9))
        # PSUM budget (8 banks): io 2 (proj chains / out-proj / grouped V
        # transposes, all 2KB slots) + sc 4 (two [128,2,512] tiles) + pv 2.
        ps_io = ctx.enter_context(tc.tile_pool(name="ps_io", bufs=2, space="PSUM"))
        ps_sc = ctx.enter_context(tc.tile_pool(name="ps_sc", bufs=2, space="PSUM"))
        ps_pv = ctx.enter_context(tc.tile_pool(name="ps_pv", bufs=2, space="PSUM"))

        # resident constants.  Only wq/bq are needed by the first matmuls;
        # issue them first so the ring backlog ahead of the q data is tiny,
        # and defer the rest behind the first input half.
        wq_sb = singles.tile([128, 8, 128], BF, name="wq_sb")
        wk_sb = singles.tile([128, 8, 128], BF, name="wk_sb")
        wv_sb = singles.tile([128, 8, 128], BF, name="wv_sb")
        wo_sb = singles.tile([128, D], BF, name="wo_sb")
        bq_sb = singles.tile([128, 1], F32, name="bq_sb")
        bk_sb = singles.tile([128, 1], F32, name="bk_sb")
        bv_sb = singles.tile([128, 1], F32, name="bv_sb")
        tri_sb = singles.tile([128, 2, 128], BF, name="tri_sb")
        id_sb = singles.tile([128, 128], BF, name="id_sb")
        nc.sync.dma_start(wq_sb[:], wq[:, :, :])
        nc.sync.dma_start(bq_sb[:], bq[:, :])

        def emit_late_consts():
            nc.sync.dma_start(wk_sb[:], wk[:, :, :])
            nc.gpsimd.dma_start(wv_sb[:], wv[:, :, :])
            nc.sync.dma_start(wo_sb[:], wo[:, :])
            nc.gpsimd.dma_start(bk_sb[:], bk[:, :])
            nc.sync.dma_start(bv_sb[:], bv[:, :])
            nc.gpsimd.dma_start(tri_sb[:], tri[:, :, :])
            nc.sync.dma_start(id_sb[:], ident[:, :])

        warm_sb = singles.tile([128, 512], BF, name="warm_sb")
        nc.vector.memset(warm_sb[:], 0.0)
        # 14 warm matmuls (~6us cold) bridge the PE across the initial input
        # stream so HAM is at full clock when the first projections land
        warm_ps = ps_io.tile([128, 512], F32, tag="proj", name="warm_ps")
        for wi in range(14):
            nc.tensor.matmul(
                warm_ps[:],
                warm_sb[:, 0:128],
                warm_sb[:],
                start=(wi == 0),
                stop=(wi == 13),
            )

        tiles = {}
        bg = deque()  # background PE/evac work items (closures)

        def bg_tick(n=1):
            for _ in range(n):
                if not bg:
                    return
                bg.popleft()()

        def alloc_batch(b):
            q_sb = qkv.tile([128, S], BF, tag="q_sb", name=f"q_sb{b}")
            k_sb = qkv.tile([128, S], BF, tag="k_sb", name=f"k_sb{b}")
            # [h0 dims (0:64) | ones (64:128) | h1 dims (128:192)]
            v_aug = qkv.tile([128, NKB, 192], BF, tag="v_aug", name=f"v_aug{b}")
            nc.vector.memset(v_aug[:, :, 64:128], 1.0)
            tiles[b] = (q_sb, k_sb, v_aug)

        def emit_projection_dmas(b, which, halves=(0, 1), st=None, nchunks=2):
            # which: 0=q, 1=k, 2=v.  One projection-half stages as a single
            # [128, 8, 1024] super-tile, loaded by two dma_starts (f 0:4 and
            # 4:8, on different queue engines) whose source runs are 8KB
            # contiguous per partition -- the matmuls for f<4 can start as
            # soon as the first dma lands (subtile deps).
            if b not in tiles:
                alloc_batch(b)
            x6 = (xq6, xk6, xv6)[which]
            if st is None:
                st = {}
            for half in halves:
                s_t = stage.tile([128, 8, 1024], BF, tag="stage")
                for c in range(nchunks):
                    fw = 8 // nchunks
                    eng = nc.sync if c % 2 == 0 else nc.gpsimd
                    eng.dma_start(
                        s_t[:, c * fw : (c + 1) * fw, :],
                        x6[b, half, :, c * fw : (c + 1) * fw, :],
                    )
                st[half] = s_t
            return st

        def emit_pst_group(b, t, v_st):
            # Grouped V transpose: 4 PE transposes into one half-bank PSUM
            # tile, then 2 merged 3D-AP copies into v_aug. Runs as one
            # background item so it occupies an io slot briefly and once.
            _, _, v_aug = tiles[b]
            pst = ps_io.tile([128, 4, 128], BF, tag="proj", name=f"pst{b}_{t}")
            for s4 in range(4):
                nc.tensor.transpose(
                    pst[:, s4, :], v_st[:, s4 * 128 : (s4 + 1) * 128], id_sb[:]
                )
            nc.vector.tensor_copy(v_aug[:, 4 * t : 4 * t + 4, 0:64], pst[:, :, 0:64])
            nc.vector.tensor_copy(
                v_aug[:, 4 * t : 4 * t + 4, 128:192], pst[:, :, 64:128]
            )

        def emit_proj_evac(b, which, t, ps, inline_pst):
            q_sb, k_sb, v_aug = tiles[b]
            if which == 0:
                # q evacs ride the ACT engine (idle during projection-heavy
                # stretches), so proj chains never wait on a DVE-queued evac
                nc.scalar.add(
                    q_sb[:, t * 512 : (t + 1) * 512], ps[:], bq_sb[:]
                )
            elif which == 1:
                nc.vector.tensor_scalar_add(
                    k_sb[:, t * 512 : (t + 1) * 512], ps[:], bk_sb[:]
                )
            else:
                v_st = vst.tile([128, 512], BF, tag="v_st")
                nc.vector.tensor_scalar_add(v_st[:], ps[:], bv_sb[:])
                if inline_pst:
                    emit_pst_group(b, t, v_st)
                else:
                    bg.append(lambda b=b, t=t, v_st=v_st: emit_pst_group(b, t, v_st))

        def emit_projection_quarter(b, which, t, s_t, inline_pst=False):
            # one [128, 8, 512] staged quarter -> one 8-matmul chain + evac
            w_sb = (wq_sb, wk_sb, wv_sb)[which]
            ps = ps_io.tile([128, 512], F32, tag="proj", name=f"psq{b}_{t}_{which}")
            for f in range(8):
                nc.tensor.matmul(
                    ps[:],
                    w_sb[:, f, :],
                    s_t[:, f, :],
                    start=(f == 0),
                    stop=(f == 7),
                )
            emit_proj_evac(b, which, t, ps, inline_pst)

        def emit_projection_mms(b, which, st, halves=(0, 1), inline_pst=False):
            # Emits the 2x2 psum groups + evac, consuming staged tiles.
            w_sb = (wq_sb, wk_sb, wv_sb)[which]

            def evac(t, ps):
                emit_proj_evac(b, which, t, ps, inline_pst)

            for half in halves:
                ps0 = ps_io.tile(
                    [128, 512], F32, tag="proj", name=f"ps{b}_{half}a_{which}"
                )
                ps1 = ps_io.tile(
                    [128, 512], F32, tag="proj", name=f"ps{b}_{half}b_{which}"
                )
                s_t = st[half]
                for f in range(8):
                    nc.tensor.matmul(
                        ps0[:],
                        w_sb[:, f, :],
                        s_t[:, f, 0:512],
                        start=(f == 0),
                        stop=(f == 7),
                    )
                    nc.tensor.matmul(
                        ps1[:],
                        w_sb[:, f, :],
                        s_t[:, f, 512:1024],
                        start=(f == 0),
                        stop=(f == 7),
                    )
                evac(half * 2, ps0)
                evac(half * 2 + 1, ps1)

        ot_pend = {}  # ob -> ot super-tile holding the even window's chunk

        def emit_po_item(ctx_t, win, ob):
            # one out-projection chunk: matmul + evac; the store DMA fires
            # once per window PAIR ([128, 1024] contiguous columns -> 2KB
            # descriptor runs, half the descriptor load).  Evacs split
            # ACT/DVE; late batches are exp-saturated on ACT, so bias their
            # evacs toward the DVE.
            po = ps_io.tile([128, 512], F32, tag="proj", name=f"po_{win}_{ob}")
            nc.tensor.matmul(
                po[:],
                wo_sb[:, ob * 128 : (ob + 1) * 128],
                ctx_t[:],
                start=True,
                stop=True,
            )
            slot = (win // 512) % 2
            if slot == 0:
                ot = outp.tile([128, 2, 512], BF, tag="ot")
                ot_pend[ob] = ot
            else:
                ot = ot_pend.pop(ob)
            use_dve = (ob % 2 == 0) if win < 2 * S else (ob % 4 != 3)
            if use_dve:
                nc.vector.tensor_copy(ot[:, slot, :], po[:])
            else:
                nc.scalar.copy(ot[:, slot, :], po[:])
            if slot == 1:
                nc.sync.dma_start(
                    out_ext[ob * 128 : (ob + 1) * 128, win - 512 : win + 512],
                    ot[:, :, :],
                )

        def emit_attention_qb(b, qb):
            # Software-pipelined: PV for block k is emitted after scores for
            # block k+3, so the PE never waits on the ACT exp of block k and
            # the PV LDWEIGHTS (which inherits the exp-done semaphore wait)
            # can prefetch into the background weight buffer.
            # Background items (out-proj chunks of the previous window,
            # grouped V transposes of the next batch) are drained one per
            # block so they fill the PE's exp-paced slack without ever
            # bunching up in the ACT queue.
            q_sb, k_sb, v_aug = tiles[b]
            nk = 4 * qb + 4  # causal: k-blocks 0 .. 4qb+3
            pv0 = ps_pv.tile([128, 512], F32, tag="pv", name=f"pv0_{b}_{qb}")
            pv1 = ps_pv.tile([128, 512], F32, tag="pv", name=f"pv1_{b}_{qb}")
            ets = {}

            def emit_scores(kblk):
                r = kblk - 4 * qb
                q_lo = max(0, r * 128)
                sc = ps_sc.tile([128, 2, 512], F32, tag="sc")
                for h in range(2):
                    nc.tensor.matmul(
                        sc[:, h, q_lo:512],
                        k_sb[h * 64 : (h + 1) * 64, kblk * 128 : (kblk + 1) * 128],
                        q_sb[h * 64 : (h + 1) * 64, qb * 512 + q_lo : (qb + 1) * 512],
                        start=True,
                        stop=True,
                        tile_position=(h * 64, 0),
                    )
                et = exps.tile([128, 2, 512], BF, tag="et")
                nc.scalar.activation(
                    et[:, :, q_lo:512], sc[:, :, q_lo:512], EXP, scale=0.125
                )
                if r >= 0:
                    nc.gpsimd.tensor_mul(
                        et[:, :, q_lo : q_lo + 128],
                        et[:, :, q_lo : q_lo + 128],
                        tri_sb[:],
                    )
                ets[kblk] = (et, q_lo)

            def emit_pv(kblk):
                et, q_lo = ets.pop(kblk)
                for h, pv in ((0, pv0), (1, pv1)):
                    nc.tensor.matmul(
                        pv[:, q_lo:512],
                        v_aug[:, kblk, h * 64 : h * 64 + 128],
                        et[:, h, q_lo:512],
                        start=(kblk == 0),
                        stop=(kblk == nk - 1),
                    )

            for kblk in range(nk):
                emit_scores(kblk)
                if kblk >= 4:
                    emit_pv(kblk - 4)
                bg_tick(1)
            emit_pv(nk - 4)
            emit_pv(nk - 3)
            emit_pv(nk - 2)
            emit_pv(nk - 1)

            # normalize: pv0 rows[0:64]=ctx~ h0, rows[64:128]=l h0 (replicated)
            #            pv1 rows[0:64]=l h1,   rows[64:128]=ctx~ h1
            # reciprocal_approx_fast only works on full-128-partition SBUF
            # tiles, so gather [l0 | l1] into lt first.
            lt = divp.tile([128, 512], F32, tag="lt")
            rec = divp.tile([128, 512], F32, tag="rec")
            ctx_t = divp.tile([128, 512], BF, tag="ctx_t")
            nc.vector.tensor_copy(lt[0:64, :], pv0[64:128, :])
            nc.vector.tensor_copy(lt[64:128, :], pv1[0:64, :])
            nc.vector.reciprocal_approx_fast(rec[:, :], lt[:, :])
            nc.vector.tensor_mul(ctx_t[0:64, :], pv0[0:64, :], rec[0:64, :])
            nc.vector.tensor_mul(ctx_t[64:128, :], pv1[64:128, :], rec[64:128, :])
            win = b * S + qb * 512
            for ob in range(8):
                bg.append(
                    lambda ctx_t=ctx_t, win=win, ob=ob: emit_po_item(ctx_t, win, ob)
                )

        # Bootstrap batch 0 by halves so attention starts as early as
        # possible: load+project half 0 of q/k/v (rows 0:1024 -> enough for
        # windows 0-1), then run windows 0-1 while half 1 loads and projects
        # as background items.  QKV(b+1) then interleaves with attention(b)
        # at qb granularity as usual.
        # round-robin the q/k/v half-0 chunk loads so each lands roughly in
        # consumption order (q chains first, then k, then v) instead of k/v
        # trailing the entire q stream in the rings
        alloc_batch(0)
        # q half-0 (2MB), then k/v quarters 0-1 from the contiguous xb0
        # layout (1MB each): window 0 starts after ~4MB has streamed,
        # window 1 after ~6MB, instead of waiting for all three halves.
        boot_q = stage.tile([128, 8, 1024], BF, tag="stage", name="boot_q")
        nc.sync.dma_start(boot_q[:, 0:4, :], xq6[0, 0, :, 0:4, :])
        nc.gpsimd.dma_start(boot_q[:, 4:8, :], xq6[0, 0, :, 4:8, :])
        bkv = {}
        qi = 0
        for t in range(2):
            for kv in range(2):
                s_t = stage.tile([128, 8, 512], BF, tag="bstage", bufs=4)
                eng = nc.sync if qi % 2 == 0 else nc.gpsimd
                qi += 1
                eng.dma_start(s_t[:], xb0[kv, t, :, :, :])
                bkv[(kv, t)] = s_t
        emit_late_consts()
        st1 = {}
        for which in range(3):
            st1[which] = emit_projection_dmas(0, which, halves=(1,))
        emit_projection_mms(0, 0, {0: boot_q}, halves=(0,), inline_pst=True)
        emit_projection_quarter(0, 1, 0, bkv[(0, 0)], inline_pst=True)
        emit_projection_quarter(0, 2, 0, bkv[(1, 0)], inline_pst=True)
        for which in range(3):
            bg.append(
                lambda which=which: emit_projection_mms(
                    0, which, st1[which], halves=(1,)
                )
            )
        for b in range(B):
            for qb in range(S // QW):
                if b == 0 and qb == 1:
                    # k/v quarter 1 (rows 512:1024) right before window 1
                    emit_projection_quarter(0, 1, 1, bkv[(0, 1)], inline_pst=True)
                    emit_projection_quarter(0, 2, 1, bkv[(1, 1)], inline_pst=True)
                if b + 1 < B and qb < 3:
                    # issue next batch's stage DMAs a full window before
                    # their matmuls so mid-run chains never wait on the rings
                    st_next = emit_projection_dmas(b + 1, qb)
                emit_attention_qb(b, qb)
                if b == 0 and qb == 1:
                    # half-1 projections must be done before window 2 reads
                    # rows 1024:2048
                    while bg:
                        bg.popleft()()
                if b + 1 < B and qb < 3:
                    emit_projection_mms(b + 1, qb, st_next)
        while bg:
            bg.popleft()()

    return nc


def _prep_in_maps(inputs):
    q = np.ascontiguousarray(inputs["query"], dtype=np.float32).reshape(R, D)
    k = np.ascontiguousarray(inputs["key"], dtype=np.float32).reshape(R, D)
    v = np.ascontiguousarray(inputs["value"], dtype=np.float32).reshape(R, D)
    Wq = np.asarray(inputs["Wq"], np.float32)
    Wk = np.asarray(inputs["Wk"], np.float32)
    Wv = np.asarray(inputs["Wv"], np.float32)
    Wo = np.asarray(inputs["Wo"], np.float32)
    bq = np.asarray(inputs["bq"], np.float32)
    bk = np.asarray(inputs["bk"], np.float32)
    bv = np.asarray(inputs["bv"], np.float32)

    def to6(x):
        # [R, D] -> [B, half, p, f, c] with x6[b,h,p,f,c] = x.T[f*128+p,
        # b*2048+h*1024+c]; per-(b,h,p) runs are 16KB contiguous.
        xT = np.ascontiguousarray(x.T).astype(bf16)  # [D, R]
        return np.ascontiguousarray(
            xT.reshape(8, 128, 4, 2, 1024).transpose(2, 3, 1, 0, 4)
        )

    xq6 = to6(q)
    xk6 = to6(k)
    xv6 = to6(v)
    # batch-0 k/v quarters 0-1 (rows 0:1024) in their own contiguous layout
    xb0 = np.ascontiguousarray(
        np.stack(
            [
                np.stack([x6[0, 0, :, :, 0:512], x6[0, 0, :, :, 512:1024]])
                for x6 in (xk6, xv6)
            ]
        )
    )
    WqT = np.ascontiguousarray(Wq.T).astype(bf16)
    WkT = np.ascontiguousarray(Wk.T).astype(bf16)
    WvT = np.ascontiguousarray(Wv.T).astype(bf16)
    WoT = np.ascontiguousarray(Wo.T).astype(bf16)
    tri_m = np.arange(128)[:, None] <= np.arange(128)[None, :]
    tri_h = np.ascontiguousarray(
        np.broadcast_to(tri_m[:, None, :], (128, 2, 128))
    ).astype(bf16)
    id_h = np.eye(128, dtype=np.float32).astype(bf16)

    in_maps = []
    for c in range(NCORES):
        sl = slice(c * 128, (c + 1) * 128)
        in_maps.append(
            {
                "xq6": xq6,
                "xk6": xk6,
                "xv6": xv6,
                "xb0": xb0,
                "wq": np.ascontiguousarray(
                    WqT[:, sl].reshape(8, 128, 128).transpose(1, 0, 2)
                ),
                "wk": np.ascontiguousarray(
                    WkT[:, sl].reshape(8, 128, 128).transpose(1, 0, 2)
                ),
                "wv": np.ascontiguousarray(
                    WvT[:, sl].reshape(8, 128, 128).transpose(1, 0, 2)
                ),
                "wo": np.ascontiguousarray(WoT[sl, :]),
                "bq": np.ascontiguousarray(bq[sl].reshape(128, 1)),
                "bk": np.ascontiguousarray(bk[sl].reshape(128, 1)),
                "bv": np.ascontiguousarray(bv[sl].reshape(128, 1)),
                "tri": tri_h,
                "ident": id_h,
            }
        )
    return in_maps


def kernel(**inputs) -> np.ndarray:
    nc = _CACHE.get("nc")
    if nc is None:
        nc = _build_program()
        nc.finalize()  # Bacc legalization (register alloc, event-sem splitting)
        _CACHE["nc"] = nc
    in_maps = _prep_in_maps(inputs)
    trace = bool(int(os.environ.get("KERNEL_TRACE", "0")))
    res = run_bass_kernel_spmd(nc, in_maps, list(range(NCORES)), trace=trace)
    _CACHE["last"] = res
    acc = res.results[0]["out"].astype(np.float32)
    for c in range(1, NCORES):
        acc += res.results[c]["out"].astype(np.float32)
    full = acc.T + np.asarray(inputs["bo"], np.float32)[None, :]
    return np.ascontiguousarray(full).reshape(B, S, D)
